# revision 29
# baseline (speedup 1.0000x reference)
"""Trainium2 Bass kernel for nn_DeepManualLSTM (3-layer LSTM, B=1024, T=48, IN=64, H=512).

Strategy: data-parallel over batch (128 rows/core x 8 cores), weights SBUF
resident. Gate GEMMs run in fp8e4m3 DoubleRow perf mode (2 k-rows packed per
matmul = 2x PE throughput); matmul operands are pre-scaled by powers of two
(h stored as 8h, W as 32W) to keep fp8 values in e4m3's normal range, and the
product scale is removed exactly by the sigmoid's fp32 `scale` parameter. The
whole cell uses only Sigmoid activations: tanh(x) = 2*sigmoid(2x) - 1 folded
into pre-scaled c-gate weight columns and fused (s - 0.5) shifts.

h reaches the transposed fp8 state tile WITHOUT a PE transpose or copy: the
elementwise tail writes 8h directly as fp8 [128,512], which viewed as fp16
[128,256] goes through the DMA XBAR transpose; the resulting interleaved
layout hT[p, 256e + 2b + i] = 8h[b, 256e + 2p + i] is consumed by DoubleRow
matmuls with a stride-reordered stationary AP (pair index i inner-stride 1,
batch stride 2) against host-interleaved weights W[256e + 2k + i, n].

Precision schedule: early timesteps use plain fp8 weights+h; late timesteps
(t >= S_SPLIT) add hi/lo fp8 splits (3 cross terms, still DoubleRow). LSTM
error decay makes early-step quantization noise nearly invisible in the final
h (rel err ~1e-2 vs the 2e-2 gate).

Per-cell engine plan (wavefront order over (t, layer), one-cell-deferred tail):
  PE:   4 PSUM banks x (state GEMM + input GEMM) fp8 DR (+ fp16 x for l=0)
  ACT:  one sigmoid over all 2048 gate cols (scale=1/256), one sigmoid(2C)
  DVE:  tanh/cell-update as tensor_scalar (4x) + tensor_tensor (2x) fp16 ops
  DMA:  XBAR transpose of the fp8 h (fp16 view)
  Pool: only for late cells: fp8 cast + lo residual
"""
import sys
import os

for _p in ("/opt/trn_rl_repo", "/root/.axon_site/_ro/trn_rl_repo"):
    if os.path.isdir(_p) and _p not in sys.path:
        sys.path.insert(0, _p)

import numpy as np
import ml_dtypes

import concourse.bass as bass
import concourse.tile as tile
from concourse import bacc, mybir
from concourse import bass_utils
from concourse.bass import ds, ts
from concourse.masks import make_identity

P = 128          # batch rows per core / SBUF partitions
T = 48           # sequence length
IN = 64          # input features
H = 512          # hidden size
L = 3            # layers
G4 = 4 * H       # gate width (2048)
NB = 4           # PSUM banks per gate row (G4 / 512)
NCORES = 8

A_H = 8.0        # h stored in fp8 as A_H * h
B_W = 32.0       # W stored in fp8 as B_W * W
SIG_SCALE = 1.0 / (A_H * B_W)   # removes the product scale, exact in fp32
S_SPLIT = 40     # timesteps >= S_SPLIT run hi+lo fp8 (3-term GEMMs)

F32 = mybir.dt.float32
F16 = mybir.dt.float16
F8 = mybir.dt.float8e4
AF = mybir.ActivationFunctionType
ALU = mybir.AluOpType
DR = mybir.MatmulPerfMode.DoubleRow

NPF8 = ml_dtypes.float8_e4m3


def _build(include_bias: bool, reps: int = 1, s_split: int = S_SPLIT,
           defer: str = "one", skew: int = 2) -> bass.Bass:
    nc = bacc.Bacc()

    xT_d = nc.dram_tensor("xT", [P, (T // 2) * P], F16, kind="ExternalInput")
    wx0_d = nc.dram_tensor("wx0", [P, G4], F16, kind="ExternalInput")
    whi_d = {}
    wlo_d = {}
    for name in ("wh0", "wh1", "wh2", "wx1", "wx2"):
        whi_d[name] = nc.dram_tensor(f"{name}_hi", [H, G4], F8, kind="ExternalInput")
        wlo_d[name] = nc.dram_tensor(f"{name}_lo", [H, G4], F8, kind="ExternalInput")
    b_d = (
        [nc.dram_tensor(f"b{l}", [1, G4], F16, kind="ExternalInput") for l in range(L)]
        if include_bias
        else None
    )
    out_d = nc.dram_tensor("hout", [P, H], F16, kind="ExternalOutput")

    with tile.TileContext(nc) as tc:
        with (
            tc.tile_pool(name="wpool", bufs=1) as wp,
            tc.tile_pool(name="state", bufs=1) as st,
            tc.tile_pool(name="work", bufs=3) as wk,
            tc.tile_pool(name="psg", bufs=2, space="PSUM") as psg,
        ):
            # ---- persistent tiles, loaded in first-use order ----------------------
            # straight chunk-major layout: tile[k, j, n] = W[128j + k, n]
            KH = H // P
            xT_t = wp.tile([P, (T // 2) * P], F16)
            nc.sync.dma_start(xT_t[:], xT_d[:])
            wx0_t = wp.tile([P, G4], F16)
            nc.sync.dma_start(wx0_t[:], wx0_d[:])
            whi = {}
            wlo = {}
            for name in ("wh0", "wx1", "wh1", "wx2", "wh2"):
                w_t = wp.tile([P, KH, G4], F8, name=name)
                nc.sync.dma_start(
                    w_t[:], whi_d[name].rearrange("(ko ki) n -> ki ko n", ki=P))
                whi[name] = w_t
            for name in ("wh0", "wx1", "wh1", "wx2", "wh2"):
                w_t = wp.tile([P, KH, G4], F8, name=f"{name}_lo")
                nc.sync.dma_start(
                    w_t[:], wlo_d[name].rearrange("(ko ki) n -> ki ko n", ki=P))
                wlo[name] = w_t

            ident = wp.tile([P, P], F16)
            make_identity(nc, ident)

            if include_bias:
                ones_t = wp.tile([1, P], F16)
                nc.vector.memset(ones_t[:], 1.0)
                b_t = []
                for l in range(L):
                    bt = wp.tile([1, G4], F16, name=f"b{l}_t")
                    nc.sync.dma_start(bt[:], b_d[l][:])
                    b_t.append(bt)

            # persistent state: C (fp16 batch-major), hT8 hi/lo (fp8 packed
            # transposed [128, 2, 256], double-buffered by t parity)
            NBUF = skew + 1
            Cs = []
            hT8 = []
            hT8lo = []
            for l in range(L):
                c_t = st.tile([P, H], F16, name=f"C{l}")
                nc.vector.memset(c_t[:], 0.0)
                Cs.append(c_t)
                hT8.append([st.tile([P, H], F8, name=f"hT8_{l}_{p}") for p in range(NBUF)])
                hT8lo.append([st.tile([P, H], F8, name=f"hT8lo_{l}_{p}") for p in range(NBUF)])

            def lhsT_of(tile8, e):
                # chunk pair (2e, 2e+1): [128, 2, 128], contiguous inner dim
                return tile8[:, 2 * e * P : (2 * e + 2) * P].rearrange(
                    "p (i b) -> p i b", i=2)

            pending = []  # [((t, l), flush_fn)] oldest first
            depth = skew  # tail deferral distance in cells
            gs = {}       # cell index -> g PSUM tile (for tail transpose space)
            ci_box = [0]

            def flush_matching(cond):
                keep = []
                for tl, fn in pending:
                    if cond(tl):
                        fn()
                    else:
                        keep.append((tl, fn))
                pending[:] = keep

            def emit_cell(t: int, l: int):
                late = t >= s_split
                par = t % NBUF

                # correctness edges: tails this cell's matmuls read from
                flush_matching(lambda tl: tl == (t - 1, l) or tl == (t, l - 1))
                # tails older than the deferral depth, at the top of the cell:
                # their sigma(2C) fills ACT's gap right after the previous
                # sigma-g and the DVE/DMA tail runs before this cell's block
                while len(pending) >= depth:
                    pending.pop(0)[1]()

                g = psg.tile([P, G4], F32, name="g", tag="gps")
                nmm = [0] * NB
                n_state = (1 if t > 0 else 0) * (3 if late else 1)
                n_input = (3 if late else 1) if l > 0 else 0
                total = (1 if include_bias else 0) + (1 if l == 0 else 0) \
                    + 2 * n_state + 2 * n_input
                totals = [total] * NB

                def mm(bank, lhsT, rhs, perf_mode=None):
                    nc.tensor.matmul(
                        g[:, ts(bank, 512)],
                        lhsT,
                        rhs,
                        start=(nmm[bank] == 0),
                        stop=(nmm[bank] == totals[bank] - 1),
                        perf_mode=perf_mode,
                        skip_group_check=True,
                    )
                    nmm[bank] += 1

                # ---- bias / x / input GEMM first (their deps are oldest) ----------
                def emit_terms(terms):
                    for tile8, w_t in terms:
                        for e in (0, 1):
                            lhsT = lhsT_of(tile8, e)
                            for n in range(NB):
                                mm(n, lhsT,
                                   w_t[:, 2 * e : 2 * e + 2, ts(n, 512)],
                                   perf_mode=DR)

                if include_bias:
                    for n in range(NB):
                        mm(n, ones_t[:], b_t[l][:, ts(n, 512)])
                if l == 0:
                    r0 = 0 if t % 2 == 0 else 64
                    xs = xT_t[r0 : r0 + IN, ts(t // 2, P)]
                    for n in range(NB):
                        mm(n, xs, wx0_t[r0 : r0 + IN, ts(n, 512)])
                input_terms = []
                if l > 0:
                    input_terms.append((hT8[l - 1][par], whi[f"wx{l}"]))
                    if late:
                        input_terms.append((hT8[l - 1][par], wlo[f"wx{l}"]))
                        input_terms.append((hT8lo[l - 1][par], whi[f"wx{l}"]))
                emit_terms(input_terms)



                # ---- state GEMM last: only ~0.9us of matmuls sit between the
                # arrival of hT8(t-1) and sigma ------------------------------------
                state_terms = []
                if t > 0:
                    sp = (t - 1) % NBUF
                    state_terms.append((hT8[l][sp], whi[f"wh{l}"]))
                    if late:
                        state_terms.append((hT8[l][sp], wlo[f"wh{l}"]))
                        state_terms.append((hT8lo[l][sp], whi[f"wh{l}"]))
                emit_terms(state_terms)

                # ---- sigma over all gates -----------------------------------------
                s = wk.tile([P, G4], F16, name="s")
                nc.scalar.activation(s[:], g[:], AF.Sigmoid, scale=SIG_SCALE)

                # cols: f 0:H, i H:2H, c 2H:3H, o 3H:4H (c-cols pre-scaled x2)
                ct = wk.tile([P, H], F16, name="ct")
                ic = wk.tile([P, H], F16, name="ic")
                fC = wk.tile([P, H], F16, name="fC")
                nc.vector.tensor_scalar(
                    ct[:], s[:, 2 * H : 3 * H], -0.5, 2.0, ALU.add, ALU.mult)
                nc.vector.tensor_mul(ic[:], ct[:], s[:, H : 2 * H])
                nc.vector.tensor_mul(fC[:], s[:, 0:H], Cs[l][:])
                nc.vector.tensor_add(Cs[l][:], ic[:], fC[:])

                # ---- deferred tail ------------------------------------------------
                st_s = wk.tile([P, H], F16, name="st_s")
                u16 = wk.tile([P, H], F16, name="u16")
                h16 = wk.tile([P, H], F16, name="h16")
                hT16 = wk.tile([P, KH, P], F16, name="hT16")
                mk_lo = late or (t == s_split - 1)
                is_last = (t == T - 1 and l == L - 1)

                def finish(l=l, par=par, s=s, st_s=st_s, u16=u16, h16=h16,
                           hT16=hT16, mk_lo=mk_lo, is_last=is_last):
                    nc.scalar.activation(st_s[:], Cs[l][:], AF.Sigmoid, scale=2.0)
                    # u = (st - 0.5) * 16 ; h16 = u * o = 8h
                    nc.vector.tensor_scalar(
                        u16[:], st_s[:], -0.5, 2.0 * A_H, ALU.add, ALU.mult)
                    nc.vector.tensor_mul(h16[:], u16[:], s[:, 3 * H :])
                    nc.sync.dma_start_transpose(hT16[:], h16[:])
                    nc.gpsimd.tensor_copy(hT8[l][par][:], hT16[:])
                    if mk_lo:
                        nc.vector.scalar_tensor_tensor(
                            hT8lo[l][par][:], hT16[:], 1.0, hT8[l][par][:],
                            ALU.mult, ALU.subtract)
                    if is_last:
                        nc.sync.dma_start(out_d[:], h16[:])

                if defer == "none":
                    finish()
                else:
                    pending.append(((t, l), finish))

            def whole_pass():
                for w in range(T + skew * (L - 1)):
                    for l in range(L):
                        t = w - skew * l
                        if 0 <= t < T:
                            emit_cell(t, l)
                while pending:
                    pending.pop(0)[1]()

            if reps > 1:
                with tc.For_i(0, reps, 1):
                    whole_pass()
            else:
                whole_pass()

    nc.finalize()
    return nc


_NC_CACHE: dict = {}
_LAST_RUN: dict = {}


def _pack_xT(x_shard: np.ndarray) -> np.ndarray:
    """[128, T, IN] -> [128, (T//2)*128] packed transposed fp16 layout."""
    xt = np.zeros((P, (T // 2) * P), dtype=np.float32)
    for t in range(T):
        r0 = 0 if t % 2 == 0 else 64
        xt[r0 : r0 + IN, (t // 2) * P : (t // 2 + 1) * P] = x_shard[:, t, :].T
    return xt.astype(np.float16)


def _prep_weights(inputs) -> dict:
    """Scale, fold tanh->sigmoid into c-columns, split hi/lo fp8.

    The fp8 tensors stay in the natural [512, 2048] row order; the kernel's
    DMA rearrange "(e k i) n -> k e i n" produces the interleaved packing that
    matches the fp8-pair DMA transpose of h.
    """
    ws = {}
    wx0 = np.asarray(inputs["Wx0"], np.float32).copy()
    wx0[:, 2 * H : 3 * H] *= 2.0
    wx0 *= A_H * B_W
    wx0d = np.zeros((P, G4), np.float32)
    wx0d[:IN] = wx0
    wx0d[IN : 2 * IN] = wx0
    ws["wx0"] = wx0d.astype(np.float16)
    for name, key in (("wh0", "Wh0"), ("wh1", "Wh1"), ("wh2", "Wh2"),
                      ("wx1", "Wx1"), ("wx2", "Wx2")):
        w = np.asarray(inputs[key], np.float32).copy()
        w[:, 2 * H : 3 * H] *= 2.0
        w *= B_W
        hi = w.astype(NPF8)
        lo = (w - hi.astype(np.float32)).astype(NPF8)
        ws[f"{name}_hi"] = hi
        ws[f"{name}_lo"] = lo
    return ws


def kernel(**inputs) -> np.ndarray:
    x = np.ascontiguousarray(np.asarray(inputs["x"], dtype=np.float32))
    B = x.shape[0]
    assert B % NCORES == 0
    Bl = B // NCORES

    fc_w = np.asarray(inputs["fc_w"], dtype=np.float32)
    fc_b = np.asarray(inputs["fc_b"], dtype=np.float32)
    bs = [np.asarray(inputs[f"b{l}"], dtype=np.float32) for l in range(L)]
    include_bias = any(np.any(b != 0) for b in bs)

    ws = _prep_weights(inputs)

    key = include_bias
    if key not in _NC_CACHE:
        _NC_CACHE[key] = _build(include_bias)
    nc = _NC_CACHE[key]
    _LAST_RUN["include_bias"] = include_bias

    in_maps = []
    for c in range(NCORES):
        m = {"xT": _pack_xT(x[c * Bl : (c + 1) * Bl])}
        m.update(ws)
        if include_bias:
            for l in range(L):
                b = bs[l].astype(np.float32).copy()
                b[2 * H : 3 * H] *= 2.0
                b *= A_H * B_W
                m[f"b{l}"] = b.astype(np.float16).reshape(1, G4)
        in_maps.append(m)

    res = bass_utils.run_bass_kernel_spmd(nc, in_maps, core_ids=list(range(NCORES)))
    _LAST_RUN["nc"] = nc
    _LAST_RUN["in_maps"] = in_maps
    outs = []
    for c in range(NCORES):
        hf = res.results[c]["hout"].astype(np.float32) / A_H  # hout = 8h
        outs.append(hf @ fc_w)
    out = np.concatenate(outs, axis=0)
    return (out + fc_b.reshape(1, -1)).astype(np.float32)
